# revision 11
# baseline (speedup 1.0000x reference)
"""AAGNN attention message-passing kernel for 8 TRN2 NeuronCores.

Math: the reference builds a dense masked attention
    att = rownorm(exp(lrelu(A*zi + diag(zj))) * A);  out = relu(z - att @ z)
Since A is 0/1 with self-loops, row i of att has only two distinct values,
so with deg(i) = rowsum(A)[i], S = (deg-1)*e_off + e_diag:
    out = relu(beta*z - alpha*(A@z)),  alpha = e_off/S, beta = 1-(e_diag-e_off)/S
One pass over A (A@[z|1] giving Az and deg) is the entire memory cost.

A compression: A is 0/1, so the host packs FOUR A-bits per byte at bit
positions 5,4,3,2. On-chip, a single DVE bitwise_and with mask
(0x20>>g) per plane yields bytes whose *bit patterns* are valid fp8e4
values c_g = {2^-3, 2^-5, 2^-6, 2^-7} (times the A bit). Each group of
16 contraction slots g has its z-slot pre-scaled by s_g = {2,8,16,32}
so c_g*s_g = 1/4 uniformly: the PSUM accumulates 0.25*[Az | deg]
exactly (up to fp8 quantization of z, ~0.6% global rel err). The A
stream drops 16 MB (f32: 32 MB) -> 2 MB per core.

Matmul: fp8 DoubleRow perf mode (2 contraction slots per instruction,
0.5 cycles/row) on the TensorEngine: lhsT = [128, 2, 65] z-slot pair,
rhs = [128, 2, 512] decompressed A, out += [65, 512] per half.

z production: bias is NOT added on-chip in the hot loop; instead
z0 = x@W^T accumulates in PSUM quads (4 slots/bank), the idle
Activation engine emits fp8 z-slots with a pure scale (af=Copy), and
the epilogue adds the analytically-folded bias term
(beta - alpha*deg)*b to the output. zi/zj keep a separate f32 path.

Sharding: core c owns output rows [c*1024, (c+1)*1024); A is symmetric
so the column stripe equals the row shard transposed. Rows are rolled
by -c0 so the SPMD graph is identical across cores. No collectives.
"""

import sys

for _p in ("/opt/trn_rl_repo",):
    if _p not in sys.path:
        sys.path.insert(0, _p)

import ml_dtypes
import numpy as np

N = 8192
IN_F = 128
OUT_F = 64
NCORES = 8
M_LOC = N // NCORES      # 1024 rows per core
NT = N // 128            # 64 contraction slots
MB = M_LOC // 128        # 8 output row-blocks per core
SLOTW = 128              # z-slot width: dual-fp8 ldweights needs all 128 cols
DEGC = OUT_F + 1         # 65: one past the deg column
AZW = OUT_F + 2          # 66: Az rows + deg row (+ pad) kept from PSUM
AUGW = OUT_F + 2         # 66: z cols + zi col + zj col
HALF = M_LOC // 2        # 512: one PSUM bank of f32
NPAIR = NT // 2          # 32 DoubleRow pairs
S_G = (2.0, 8.0, 16.0, 32.0)          # z-slot scales per 16-slot group
MASK8 = (0x20, 0x10, 0x08, 0x04)      # bit-plane masks (fp8e4 c_g = 0.25/s_g)

_CACHE = {}


def _emit(tc, nc, bass, mybir, make_identity, apack, inputst, park, out_ext):
    f32 = mybir.dt.float32
    bf16 = mybir.dt.bfloat16
    f8 = mybir.dt.float8e4
    u16 = mybir.dt.uint16
    Act = mybir.ActivationFunctionType
    Alu = mybir.AluOpType
    DR = mybir.MatmulPerfMode.DoubleRow

    consts = tc.alloc_tile_pool(name="consts", bufs=1)
    decp = tc.alloc_tile_pool(name="decp", bufs=16)
    ep1 = tc.alloc_tile_pool(name="ep1", bufs=2)
    ep64 = tc.alloc_tile_pool(name="ep64", bufs=4)
    outp = tc.alloc_tile_pool(name="outp", bufs=3)

    # ---- input streaming, issued before anything else ---------------------
    park_sb = consts.tile([128, 195], f32)
    nc.sync.dma_start(out=park_sb, in_=park[:, :])
    in_all = consts.tile([IN_F, N], bf16)
    apack_sb = consts.tile([128, N], u16)   # 4 A-bits per byte
    nc.sync.dma_start(out=in_all[:, 0:512], in_=inputst[:, 0:512])
    nc.sync.dma_start(out=apack_sb[:, 0:2048], in_=apack[:, 0:2048])
    nc.sync.dma_start(out=in_all[:, 512:8192], in_=inputst[:, 512:8192])
    nc.sync.dma_start(out=apack_sb[:, 2048:8192], in_=apack[:, 2048:8192])

    # ---- constants / setup ------------------------------------------------
    identity = consts.tile([128, 128], f32)
    make_identity(nc, identity)
    ones1 = consts.tile([1, 128], f32)
    nc.gpsimd.memset(ones1, 1.0)
    warm = consts.tile([1, 1], f32)
    nc.scalar.activation(warm, ones1[0:1, 0:1], Act.Copy)

    wt_aug = consts.tile([128, AUGW], bf16)   # [W^T | w1 | w2]
    bias_bc = consts.tile([128, AUGW], f32)   # [b bcast | a1.b | a2.b]
    z8 = consts.tile([128, NT * SLOTW], f8)   # fp8 z slots [s*z | s]
    z_loc32 = consts.tile([128, MB * OUT_F], f32)
    zi_loc = consts.tile([128, 2 * MB], f32)  # local [zi | zj] per block

    # z8 pad + scaled-ones presets: no input deps, so DVE runs these at t=0.
    # Bulk pad (bytes 66..127 of each slot) as u16 for the 2x DVE mode.
    z8v = z8.rearrange("p (t w) -> p t w", w=SLOTW)
    nc.vector.memset(z8v[:, :, OUT_F + 2:SLOTW].bitcast(u16), 0)
    nc.vector.memset(z8v[:, :, DEGC:OUT_F + 2], 0.0)
    for g in range(4):
        nc.vector.memset(z8v[:, g * 16:(g + 1) * 16, OUT_F:DEGC], S_G[g])

    pre_psum = tc.alloc_tile_pool(name="pre_psum", bufs=1, space="PSUM")
    nc.vector.tensor_copy(out=wt_aug[:, 0:OUT_F], in_=park_sb[:, 0:OUT_F])
    w12_ps = pre_psum.tile([128, 2], f32)
    nc.tensor.matmul(w12_ps, lhsT=park_sb[0:OUT_F, 67:195],
                     rhs=park_sb[0:OUT_F, 65:67], start=True, stop=True)
    nc.vector.tensor_copy(out=wt_aug[:, OUT_F:AUGW], in_=w12_ps)

    brow_ps = pre_psum.tile([1, OUT_F], f32)
    nc.tensor.transpose(brow_ps, park_sb[0:OUT_F, 64:65],
                        identity[0:OUT_F, 0:OUT_F])
    ab_ps = pre_psum.tile([1, 2], f32)
    nc.tensor.matmul(ab_ps, lhsT=park_sb[0:OUT_F, 64:65],
                     rhs=park_sb[0:OUT_F, 65:67], start=True, stop=True)
    rhs_row = consts.tile([1, AUGW], f32)
    nc.vector.tensor_copy(out=rhs_row[0:1, 0:OUT_F], in_=brow_ps)
    nc.vector.tensor_copy(out=rhs_row[0:1, OUT_F:AUGW], in_=ab_ps)
    bias_ps = pre_psum.tile([128, AUGW], f32)
    nc.tensor.matmul(bias_ps, lhsT=ones1, rhs=rhs_row, start=True, stop=True)
    nc.vector.tensor_copy(out=bias_bc, in_=bias_ps)
    pre_psum.release()

    # ---- fused z production + decompress + message-passing matmul --------
    psum2 = tc.alloc_tile_pool(name="psum2", bufs=1, space="PSUM")
    acc_t = [psum2.tile([128, HALF], f32, tag=f"acct{h}", name=f"acct{h}")
             for h in range(2)]
    zpsum = tc.alloc_tile_pool(name="zpsum", bufs=3, space="PSUM")

    def z_quad(Q):
        zq = zpsum.tile([128, 4 * AUGW], f32, tag="zq", name=f"zq{Q}")
        for j in range(4):
            kb = 4 * Q + j
            nc.tensor.matmul(zq[:, j * AUGW:(j + 1) * AUGW],
                             lhsT=in_all[:, kb * 128:(kb + 1) * 128],
                             rhs=wt_aug, start=True, stop=True)
        zqv = zq.rearrange("p (j w) -> p j w", w=AUGW)
        nc.scalar.activation(z8v[:, 4 * Q:4 * Q + 4, 0:OUT_F],
                             zqv[:, :, 0:OUT_F], Act.Copy,
                             scale=float(S_G[Q // 4]))
        if Q < 2:
            zl = z_loc32.rearrange("p (j w) -> p j w", w=OUT_F)
            nc.scalar.activation(zl[:, 4 * Q:4 * Q + 4, :],
                                 zqv[:, :, 0:OUT_F], Act.Copy)
            ziv = zi_loc.rearrange("p (m t) -> p m t", t=2)
            for c in range(2):
                nc.scalar.activation(
                    ziv[:, 4 * Q:4 * Q + 4, c:c + 1],
                    zqv[:, :, OUT_F + c:OUT_F + c + 1], Act.Identity,
                    bias=bias_bc[:, OUT_F + c:OUT_F + c + 1])

    z_quad(0)
    z_quad(1)
    for u in range(16):
        if u + 2 < 16:
            z_quad(u + 2)
        g = u // 4
        q = (4 * u) % 16
        dec = decp.tile([128, 4 * M_LOC], f8, tag="dec", name=f"dec{u}")
        nc.vector.tensor_scalar(
            out=dec[:, :].bitcast(u16),
            in0=apack_sb[:, q * 512:q * 512 + 2048],
            scalar1=MASK8[g] * 0x101, scalar2=None, op0=Alu.bitwise_and)
        decv = dec.rearrange("p (k m) -> p k m", m=M_LOC)
        for i in range(2):
            t = 2 * u + i
            zpair = z8v[:, 2 * t:2 * t + 2, :]
            for h in range(2):
                nc.tensor.matmul(acc_t[h], lhsT=zpair,
                                 rhs=decv[:, 2 * i:2 * i + 2,
                                          h * HALF:(h + 1) * HALF],
                                 start=(t == 0), stop=(t == NPAIR - 1),
                                 perf_mode=DR)

    # ---- batched attention-coefficient math (zi/zj part, runs early) -----
    zis = zi_loc.rearrange("p (m t) -> p m t", t=2)[:, :, 0:1]  # [128, 8, 1]
    zjs = zi_loc.rearrange("p (m t) -> p m t", t=2)[:, :, 1:2]
    s8 = ep1.tile([128, MB], f32, tag="s8")
    nc.vector.tensor_add(s8, zis, zjs)
    t8 = ep1.tile([128, MB], f32, tag="t8")
    nc.vector.tensor_scalar_mul(t8, zis, 0.01)
    l8 = ep1.tile([128, MB], f32, tag="l8")
    nc.vector.tensor_max(l8, zis, t8)
    eoff8 = ep1.tile([128, MB], f32, tag="eoff8")
    nc.scalar.activation(eoff8, l8, Act.Exp)
    t8b = ep1.tile([128, MB], f32, tag="t8b")
    nc.vector.tensor_scalar_mul(t8b, s8, 0.01)
    l8b = ep1.tile([128, MB], f32, tag="l8b")
    nc.vector.tensor_max(l8b, s8, t8b)
    ediag8 = ep1.tile([128, MB], f32, tag="ediag8")
    nc.scalar.activation(ediag8, l8b, Act.Exp)
    gd8 = ep1.tile([128, MB], f32, tag="gd8")
    nc.vector.tensor_sub(gd8, ediag8, eoff8)

    zpsum.release()

    # copy 0.25*[Az|deg]^T to SBUF; deg row first so the coefficient
    # chain (transposes + S/alpha/beta math) starts before the bulk copy
    azt_sb = consts.tile([AZW, M_LOC], f32)
    for h in range(2):
        nc.scalar.activation(azt_sb[OUT_F:DEGC, h * HALF:(h + 1) * HALF],
                             acc_t[h][OUT_F:DEGC, :], Act.Copy)
    for h in range(2):
        nc.scalar.activation(azt_sb[0:OUT_F, h * HALF:(h + 1) * HALF],
                             acc_t[h][0:OUT_F, :], Act.Copy)

    # deg row -> node-on-partition via single-column PE transposes
    tpsum = tc.alloc_tile_pool(name="tpsum", bufs=4, space="PSUM")
    dpsum = tc.alloc_tile_pool(name="dpsum", bufs=1, space="PSUM")
    tp_deg = dpsum.tile([128, MB], f32)   # = 0.25*deg
    for mb in range(MB):
        nc.tensor.transpose(tp_deg[:, mb:mb + 1],
                            azt_sb[OUT_F:DEGC, mb * 128:(mb + 1) * 128],
                            identity[OUT_F:DEGC, OUT_F:DEGC])
    dm8 = ep1.tile([128, MB], f32, tag="dm8")   # deg - 1
    nc.vector.tensor_scalar(out=dm8, in0=tp_deg, scalar1=4.0, scalar2=-1.0,
                            op0=Alu.mult, op1=Alu.add)
    S8 = ep1.tile([128, MB], f32, tag="S8")
    nc.vector.tensor_tensor(out=S8, in0=dm8, in1=eoff8, op=Alu.mult)
    nc.vector.tensor_add(S8, S8, ediag8)
    rS8 = ep1.tile([128, MB], f32, tag="rS8")
    nc.vector.reciprocal(rS8, S8)
    alpha8 = ep1.tile([128, MB], f32, tag="alpha8")   # = 4*alpha
    nc.vector.scalar_tensor_tensor(alpha8, in0=eoff8, scalar=4.0, in1=rS8,
                                   op0=Alu.mult, op1=Alu.mult)
    gamma8 = ep1.tile([128, MB], f32, tag="gamma8")
    nc.vector.tensor_mul(gamma8, gd8, rS8)
    beta8 = ep1.tile([128, MB], f32, tag="beta8")
    nc.vector.tensor_scalar(
        out=beta8, in0=gamma8, scalar1=-1.0, scalar2=1.0,
        op0=Alu.mult, op1=Alu.add)
    adeg8 = ep1.tile([128, MB], f32, tag="adeg8")   # alpha*deg
    nc.vector.tensor_mul(adeg8, alpha8, tp_deg)
    c8 = ep1.tile([128, MB], f32, tag="c8")   # beta - alpha*deg
    nc.vector.tensor_sub(c8, beta8, adeg8)

    # ---- per-block: transpose Az back, combine + bias term, relu ---------
    osb_all = outp.tile([128, MB * OUT_F], f32)
    for mb in range(MB):
        tp = tpsum.tile([128, AZW], f32, tag="tp", name=f"tp{mb}")
        nc.tensor.transpose(tp, azt_sb[:, mb * 128:(mb + 1) * 128],
                            identity[:AZW, :AZW])
        zloc = z_loc32[:, mb * OUT_F:(mb + 1) * OUT_F]
        t1 = ep64.tile([128, OUT_F], f32, tag="t1")
        nc.scalar.activation(t1, tp[:, 0:OUT_F], Act.Copy,
                             scale=alpha8[:, mb:mb + 1])
        h1 = ep64.tile([128, OUT_F], f32, tag="h1")
        nc.vector.scalar_tensor_tensor(
            h1, in0=zloc, scalar=beta8[:, mb:mb + 1], in1=t1,
            op0=Alu.mult, op1=Alu.subtract)
        h2 = ep64.tile([128, OUT_F], f32, tag="h2")
        nc.vector.scalar_tensor_tensor(
            h2, in0=bias_bc[:, 0:OUT_F], scalar=c8[:, mb:mb + 1], in1=h1,
            op0=Alu.mult, op1=Alu.add)
        nc.scalar.activation(osb_all[:, mb * OUT_F:(mb + 1) * OUT_F], h2,
                             Act.Relu)
        if mb == 3:
            nc.sync.dma_start(out=out_ext[:, 0:4 * OUT_F],
                              in_=osb_all[:, 0:4 * OUT_F])
    nc.sync.dma_start(out=out_ext[:, 4 * OUT_F:], in_=osb_all[:, 4 * OUT_F:])

    dpsum.release()
    tpsum.release()
    psum2.release()
    for p in (outp, ep64, ep1, decp, consts):
        p.release()


def _build():
    import concourse.bass as bass
    import concourse.mybir as mybir
    import concourse.tile as tile
    from concourse import bacc
    from concourse.masks import make_identity

    f32 = mybir.dt.float32
    nc = bacc.Bacc("TRN2", target_bir_lowering=False, debug=False)
    # apack: 4 A-bits per byte at bit positions 5..2; byte[p, q*1024+m] holds
    # A_rolled[(g*16+q)*128 + p, m] at bit (5-g). Viewed as uint16 on device.
    apack = nc.declare_dram_parameter("apack", [128, N], mybir.dt.uint16,
                                      isOutput=False)
    inputst = nc.declare_dram_parameter("inputst", [IN_F, N],
                                        mybir.dt.bfloat16, isOutput=False)
    # park: [W^T | b | a1 | a2] in one tensor (single param DMA)
    park = nc.declare_dram_parameter("park", [128, 195], f32,
                                     isOutput=False)
    out_ext = nc.declare_dram_parameter("out", [128, MB * OUT_F], f32,
                                        isOutput=True)

    with tile.TileContext(nc) as tc:
        _emit(tc, nc, bass, mybir, make_identity, apack, inputst, park,
              out_ext)
    nc.compile()
    return nc


def _in_maps(inputs):
    A = np.asarray(inputs["A"], dtype=np.float32)
    X = np.asarray(inputs["inputs"], dtype=np.float32)
    W = np.asarray(inputs["W"], dtype=np.float32)
    b = np.asarray(inputs["b"], dtype=np.float32).reshape(OUT_F)
    a1 = np.asarray(inputs["a1"], dtype=np.float32).reshape(OUT_F)
    a2 = np.asarray(inputs["a2"], dtype=np.float32).reshape(OUT_F)

    park = np.zeros((128, 195), np.float32)
    park[:, 0:OUT_F] = W.T
    park[0:OUT_F, 64] = b
    park[0:OUT_F, 65] = a1
    park[0:OUT_F, 66] = a2
    park[0:OUT_F, 67:195] = W

    wbits = np.array(MASK8, np.uint8).reshape(4, 1, 1, 1)
    maps = []
    for c in range(NCORES):
        c0 = c * M_LOC
        stripe = A[:, c0:c0 + M_LOC]
        ash = np.concatenate([stripe[c0:], stripe[:c0]], axis=0)
        R = (ash.reshape(4, 16, 128, M_LOC) != 0)
        B = (R * wbits).sum(axis=0).astype(np.uint8)      # [16, 128, 1024]
        B = np.ascontiguousarray(B.transpose(1, 0, 2).reshape(128, 2 * N))
        xr = np.concatenate([X[c0:], X[:c0]], axis=0)
        maps.append({
            "apack": B.view(np.uint16),
            "inputst": np.ascontiguousarray(xr.T).astype(ml_dtypes.bfloat16),
            "park": park,
        })
    return maps


def _run(inputs, trace=False):
    from concourse.bass_utils import run_bass_kernel_spmd

    if "nc" not in _CACHE:
        _CACHE["nc"] = _build()
    nc = _CACHE["nc"]
    res = run_bass_kernel_spmd(nc, _in_maps(inputs), list(range(NCORES)),
                               trace=trace)
    parts = []
    for c in range(NCORES):
        packed = res.results[c]["out"]  # [128, MB*OUT_F], block-interleaved
        parts.append(packed.reshape(128, MB, OUT_F).transpose(1, 0, 2)
                     .reshape(M_LOC, OUT_F))
    out = np.concatenate(parts, axis=0)
    return out, res


def kernel(**inputs):
    out, _ = _run(inputs, trace=False)
    return out


def kernel_traced(**inputs):
    out, res = _run(inputs, trace=True)
    return out, res


# revision 12
# speedup vs baseline: 1.0234x; 1.0234x over previous
"""AAGNN attention message-passing kernel for 8 TRN2 NeuronCores.

Math: the reference builds a dense masked attention
    att = rownorm(exp(lrelu(A*zi + diag(zj))) * A);  out = relu(z - att @ z)
Since A is 0/1 with self-loops, row i of att has only two distinct values,
so with deg(i) = rowsum(A)[i], S = (deg-1)*e_off + e_diag:
    out = relu(beta*z - alpha*(A@z)),  alpha = e_off/S, beta = 1-(e_diag-e_off)/S
One pass over A (A@[z|1] giving Az and deg) is the entire memory cost.

A compression: A is 0/1, so the host packs FOUR A-bits per byte at bit
positions 5,4,3,2. On-chip, a single DVE bitwise_and with mask
(0x20>>g) per plane yields bytes whose *bit patterns* are valid fp8e4
values c_g = {2^-3, 2^-5, 2^-6, 2^-7} (times the A bit). Each group of
16 contraction slots g has its z-slot pre-scaled by s_g = {2,8,16,32}
so c_g*s_g = 1/4 uniformly: the PSUM accumulates 0.25*[Az | deg]
exactly (up to fp8 quantization of z, ~0.6% global rel err). The A
stream drops 16 MB (f32: 32 MB) -> 2 MB per core.

Matmul: fp8 DoubleRow perf mode (2 contraction slots per instruction,
0.5 cycles/row) on the TensorEngine: lhsT = [128, 2, 65] z-slot pair,
rhs = [128, 2, 512] decompressed A, out += [65, 512] per half.

z production: bias is NOT added on-chip in the hot loop; instead
z0 = x@W^T accumulates in PSUM quads (4 slots/bank), the idle
Activation engine emits fp8 z-slots with a pure scale (af=Copy), and
the epilogue adds the analytically-folded bias term
(beta - alpha*deg)*b to the output. zi/zj keep a separate f32 path.

Sharding: core c owns output rows [c*1024, (c+1)*1024); A is symmetric
so the column stripe equals the row shard transposed. Rows are rolled
by -c0 so the SPMD graph is identical across cores. No collectives.
"""

import sys

for _p in ("/opt/trn_rl_repo",):
    if _p not in sys.path:
        sys.path.insert(0, _p)

import ml_dtypes
import numpy as np

N = 8192
IN_F = 128
OUT_F = 64
NCORES = 8
M_LOC = N // NCORES      # 1024 rows per core
NT = N // 128            # 64 contraction slots
MB = M_LOC // 128        # 8 output row-blocks per core
SLOTW = 128              # z-slot width: dual-fp8 ldweights needs all 128 cols
DEGC = OUT_F + 1         # 65: one past the deg column
AZW = OUT_F + 2          # 66: Az rows + deg row (+ pad) kept from PSUM
AUGW = OUT_F + 2         # 66: z cols + zi col + zj col
HALF = M_LOC // 2        # 512: one PSUM bank of f32
NPAIR = NT // 2          # 32 DoubleRow pairs
S_G = (2.0, 8.0, 16.0, 32.0)          # z-slot scales per 16-slot group
MASK8 = (0x20, 0x10, 0x08, 0x04)      # bit-plane masks (fp8e4 c_g = 0.25/s_g)

_CACHE = {}


def _emit(tc, nc, bass, mybir, make_identity, apack, inputst, park, out_ext):
    f32 = mybir.dt.float32
    bf16 = mybir.dt.bfloat16
    f8 = mybir.dt.float8e4
    u16 = mybir.dt.uint16
    Act = mybir.ActivationFunctionType
    Alu = mybir.AluOpType
    DR = mybir.MatmulPerfMode.DoubleRow

    consts = tc.alloc_tile_pool(name="consts", bufs=1)
    decp = tc.alloc_tile_pool(name="decp", bufs=32)
    ep1 = tc.alloc_tile_pool(name="ep1", bufs=2)
    ep64 = tc.alloc_tile_pool(name="ep64", bufs=4)
    outp = tc.alloc_tile_pool(name="outp", bufs=3)

    # ---- input streaming, issued before anything else ---------------------
    park_sb = consts.tile([128, 195], f32)
    nc.sync.dma_start(out=park_sb, in_=park[:, :])
    in_all = consts.tile([IN_F, N], bf16)
    apack_sb = consts.tile([128, N], u16)   # 4 A-bits per byte
    nc.sync.dma_start(out=in_all[:, 0:512], in_=inputst[:, 0:512])
    nc.sync.dma_start(out=apack_sb[:, 0:2048], in_=apack[:, 0:2048])
    nc.sync.dma_start(out=in_all[:, 512:8192], in_=inputst[:, 512:8192])
    nc.sync.dma_start(out=apack_sb[:, 2048:8192], in_=apack[:, 2048:8192])

    # ---- constants / setup ------------------------------------------------
    identity = consts.tile([128, 128], f32)
    make_identity(nc, identity)
    ones1 = consts.tile([1, 128], f32)
    nc.gpsimd.memset(ones1, 1.0)
    warm = consts.tile([1, 1], f32)
    nc.scalar.activation(warm, ones1[0:1, 0:1], Act.Copy)

    wt_aug = consts.tile([128, AUGW], bf16)   # [W^T | w1 | w2]
    bias_bc = consts.tile([128, AUGW], f32)   # [b bcast | a1.b | a2.b]
    z8 = consts.tile([128, NT * SLOTW], f8)   # fp8 z slots [s*z | s]
    z_loc32 = consts.tile([128, MB * OUT_F], f32)
    zi_loc = consts.tile([128, 2 * MB], f32)  # local [zi | zj] per block

    # z8 pad + scaled-ones presets: no input deps, so DVE runs these at t=0.
    # Bulk pad (bytes 66..127 of each slot) as u16 for the 2x DVE mode.
    z8v = z8.rearrange("p (t w) -> p t w", w=SLOTW)
    nc.vector.memset(z8v[:, :, OUT_F + 2:SLOTW].bitcast(u16), 0)
    nc.vector.memset(z8v[:, :, DEGC:OUT_F + 2], 0.0)
    for g in range(4):
        nc.vector.memset(z8v[:, g * 16:(g + 1) * 16, OUT_F:DEGC], S_G[g])

    pre_psum = tc.alloc_tile_pool(name="pre_psum", bufs=1, space="PSUM")
    nc.vector.tensor_copy(out=wt_aug[:, 0:OUT_F], in_=park_sb[:, 0:OUT_F])
    w12_ps = pre_psum.tile([128, 2], f32)
    nc.tensor.matmul(w12_ps, lhsT=park_sb[0:OUT_F, 67:195],
                     rhs=park_sb[0:OUT_F, 65:67], start=True, stop=True)
    nc.vector.tensor_copy(out=wt_aug[:, OUT_F:AUGW], in_=w12_ps)

    brow_ps = pre_psum.tile([1, OUT_F], f32)
    nc.tensor.transpose(brow_ps, park_sb[0:OUT_F, 64:65],
                        identity[0:OUT_F, 0:OUT_F])
    ab_ps = pre_psum.tile([1, 2], f32)
    nc.tensor.matmul(ab_ps, lhsT=park_sb[0:OUT_F, 64:65],
                     rhs=park_sb[0:OUT_F, 65:67], start=True, stop=True)
    rhs_row = consts.tile([1, AUGW], f32)
    nc.vector.tensor_copy(out=rhs_row[0:1, 0:OUT_F], in_=brow_ps)
    nc.vector.tensor_copy(out=rhs_row[0:1, OUT_F:AUGW], in_=ab_ps)
    bias_ps = pre_psum.tile([128, AUGW], f32)
    nc.tensor.matmul(bias_ps, lhsT=ones1, rhs=rhs_row, start=True, stop=True)
    nc.vector.tensor_copy(out=bias_bc, in_=bias_ps)
    pre_psum.release()

    # ---- fused z production + decompress + message-passing matmul --------
    psum2 = tc.alloc_tile_pool(name="psum2", bufs=1, space="PSUM")
    acc_t = [psum2.tile([128, HALF], f32, tag=f"acct{h}", name=f"acct{h}")
             for h in range(2)]
    zpsum = tc.alloc_tile_pool(name="zpsum", bufs=3, space="PSUM")

    def z_quad(Q):
        zq = zpsum.tile([128, 4 * AUGW], f32, tag="zq", name=f"zq{Q}")
        for j in range(4):
            kb = 4 * Q + j
            nc.tensor.matmul(zq[:, j * AUGW:(j + 1) * AUGW],
                             lhsT=in_all[:, kb * 128:(kb + 1) * 128],
                             rhs=wt_aug, start=True, stop=True)
        zqv = zq.rearrange("p (j w) -> p j w", w=AUGW)
        nc.scalar.activation(z8v[:, 4 * Q:4 * Q + 4, 0:OUT_F],
                             zqv[:, :, 0:OUT_F], Act.Copy,
                             scale=float(S_G[Q // 4]))
        if Q < 2:
            zl = z_loc32.rearrange("p (j w) -> p j w", w=OUT_F)
            nc.scalar.activation(zl[:, 4 * Q:4 * Q + 4, :],
                                 zqv[:, :, 0:OUT_F], Act.Copy)
            ziv = zi_loc.rearrange("p (m t) -> p m t", t=2)
            for c in range(2):
                nc.scalar.activation(
                    ziv[:, 4 * Q:4 * Q + 4, c:c + 1],
                    zqv[:, :, OUT_F + c:OUT_F + c + 1], Act.Identity,
                    bias=bias_bc[:, OUT_F + c:OUT_F + c + 1])

    z_quad(0)
    z_quad(1)
    for t in range(NPAIR):
        if t % 2 == 0 and t // 2 + 2 < 16:
            z_quad(t // 2 + 2)
        g = t // 8
        q = (2 * t) % 16
        dec = decp.tile([128, 2 * M_LOC], f8, tag="dec", name=f"dec{t}")
        nc.vector.tensor_scalar(
            out=dec[:, :].bitcast(u16),
            in0=apack_sb[:, q * 512:q * 512 + 1024],
            scalar1=MASK8[g] * 0x101, scalar2=None, op0=Alu.bitwise_and)
        zpair = z8v[:, 2 * t:2 * t + 2, :]
        decv = dec.rearrange("p (k m) -> p k m", m=M_LOC)
        for h in range(2):
            nc.tensor.matmul(acc_t[h], lhsT=zpair,
                             rhs=decv[:, :, h * HALF:(h + 1) * HALF],
                             start=(t == 0), stop=(t == NPAIR - 1),
                             perf_mode=DR)

    # ---- batched attention-coefficient math (zi/zj part, runs early) -----
    zis = zi_loc.rearrange("p (m t) -> p m t", t=2)[:, :, 0:1]  # [128, 8, 1]
    zjs = zi_loc.rearrange("p (m t) -> p m t", t=2)[:, :, 1:2]
    s8 = ep1.tile([128, MB], f32, tag="s8")
    nc.vector.tensor_add(s8, zis, zjs)
    t8 = ep1.tile([128, MB], f32, tag="t8")
    nc.vector.tensor_scalar_mul(t8, zis, 0.01)
    l8 = ep1.tile([128, MB], f32, tag="l8")
    nc.vector.tensor_max(l8, zis, t8)
    eoff8 = ep1.tile([128, MB], f32, tag="eoff8")
    nc.scalar.activation(eoff8, l8, Act.Exp)
    t8b = ep1.tile([128, MB], f32, tag="t8b")
    nc.vector.tensor_scalar_mul(t8b, s8, 0.01)
    l8b = ep1.tile([128, MB], f32, tag="l8b")
    nc.vector.tensor_max(l8b, s8, t8b)
    ediag8 = ep1.tile([128, MB], f32, tag="ediag8")
    nc.scalar.activation(ediag8, l8b, Act.Exp)
    gd8 = ep1.tile([128, MB], f32, tag="gd8")
    nc.vector.tensor_sub(gd8, ediag8, eoff8)

    zpsum.release()

    # copy 0.25*[Az|deg]^T to SBUF; deg row first so the coefficient
    # chain (transposes + S/alpha/beta math) starts before the bulk copy
    azt_sb = consts.tile([AZW, M_LOC], f32)
    for h in range(2):
        nc.scalar.activation(azt_sb[OUT_F:DEGC, h * HALF:(h + 1) * HALF],
                             acc_t[h][OUT_F:DEGC, :], Act.Copy)
    for h in range(2):
        nc.scalar.activation(azt_sb[0:OUT_F, h * HALF:(h + 1) * HALF],
                             acc_t[h][0:OUT_F, :], Act.Copy)

    # deg row -> node-on-partition via single-column PE transposes
    tpsum = tc.alloc_tile_pool(name="tpsum", bufs=4, space="PSUM")
    dpsum = tc.alloc_tile_pool(name="dpsum", bufs=1, space="PSUM")
    tp_deg = dpsum.tile([128, MB], f32)   # = 0.25*deg
    for mb in range(MB):
        nc.tensor.transpose(tp_deg[:, mb:mb + 1],
                            azt_sb[OUT_F:DEGC, mb * 128:(mb + 1) * 128],
                            identity[OUT_F:DEGC, OUT_F:DEGC])
    dm8 = ep1.tile([128, MB], f32, tag="dm8")   # deg - 1
    nc.vector.tensor_scalar(out=dm8, in0=tp_deg, scalar1=4.0, scalar2=-1.0,
                            op0=Alu.mult, op1=Alu.add)
    S8 = ep1.tile([128, MB], f32, tag="S8")
    nc.vector.tensor_tensor(out=S8, in0=dm8, in1=eoff8, op=Alu.mult)
    nc.vector.tensor_add(S8, S8, ediag8)
    rS8 = ep1.tile([128, MB], f32, tag="rS8")
    nc.vector.reciprocal(rS8, S8)
    alpha8 = ep1.tile([128, MB], f32, tag="alpha8")   # = 4*alpha
    nc.vector.scalar_tensor_tensor(alpha8, in0=eoff8, scalar=4.0, in1=rS8,
                                   op0=Alu.mult, op1=Alu.mult)
    gamma8 = ep1.tile([128, MB], f32, tag="gamma8")
    nc.vector.tensor_mul(gamma8, gd8, rS8)
    beta8 = ep1.tile([128, MB], f32, tag="beta8")
    nc.vector.tensor_scalar(
        out=beta8, in0=gamma8, scalar1=-1.0, scalar2=1.0,
        op0=Alu.mult, op1=Alu.add)
    adeg8 = ep1.tile([128, MB], f32, tag="adeg8")   # alpha*deg
    nc.vector.tensor_mul(adeg8, alpha8, tp_deg)
    c8 = ep1.tile([128, MB], f32, tag="c8")   # beta - alpha*deg
    nc.vector.tensor_sub(c8, beta8, adeg8)

    # ---- per-block: transpose Az back, combine + bias term, relu ---------
    osb_all = outp.tile([128, MB * OUT_F], f32)
    for mb in range(MB):
        tp = tpsum.tile([128, AZW], f32, tag="tp", name=f"tp{mb}")
        nc.tensor.transpose(tp, azt_sb[:, mb * 128:(mb + 1) * 128],
                            identity[:AZW, :AZW])
        zloc = z_loc32[:, mb * OUT_F:(mb + 1) * OUT_F]
        t1 = ep64.tile([128, OUT_F], f32, tag="t1")
        nc.scalar.activation(t1, tp[:, 0:OUT_F], Act.Copy,
                             scale=alpha8[:, mb:mb + 1])
        h1 = ep64.tile([128, OUT_F], f32, tag="h1")
        nc.vector.scalar_tensor_tensor(
            h1, in0=zloc, scalar=beta8[:, mb:mb + 1], in1=t1,
            op0=Alu.mult, op1=Alu.subtract)
        h2 = ep64.tile([128, OUT_F], f32, tag="h2")
        nc.vector.scalar_tensor_tensor(
            h2, in0=bias_bc[:, 0:OUT_F], scalar=c8[:, mb:mb + 1], in1=h1,
            op0=Alu.mult, op1=Alu.add)
        nc.scalar.activation(osb_all[:, mb * OUT_F:(mb + 1) * OUT_F], h2,
                             Act.Relu)
        if mb == 3:
            nc.sync.dma_start(out=out_ext[:, 0:4 * OUT_F],
                              in_=osb_all[:, 0:4 * OUT_F])
    nc.sync.dma_start(out=out_ext[:, 4 * OUT_F:], in_=osb_all[:, 4 * OUT_F:])

    dpsum.release()
    tpsum.release()
    psum2.release()
    for p in (outp, ep64, ep1, decp, consts):
        p.release()


def _build():
    import concourse.bass as bass
    import concourse.mybir as mybir
    import concourse.tile as tile
    from concourse import bacc
    from concourse.masks import make_identity

    f32 = mybir.dt.float32
    nc = bacc.Bacc("TRN2", target_bir_lowering=False, debug=False)
    # apack: 4 A-bits per byte at bit positions 5..2; byte[p, q*1024+m] holds
    # A_rolled[(g*16+q)*128 + p, m] at bit (5-g). Viewed as uint16 on device.
    apack = nc.declare_dram_parameter("apack", [128, N], mybir.dt.uint16,
                                      isOutput=False)
    inputst = nc.declare_dram_parameter("inputst", [IN_F, N],
                                        mybir.dt.bfloat16, isOutput=False)
    # park: [W^T | b | a1 | a2] in one tensor (single param DMA)
    park = nc.declare_dram_parameter("park", [128, 195], f32,
                                     isOutput=False)
    out_ext = nc.declare_dram_parameter("out", [128, MB * OUT_F], f32,
                                        isOutput=True)

    with tile.TileContext(nc) as tc:
        _emit(tc, nc, bass, mybir, make_identity, apack, inputst, park,
              out_ext)
    nc.compile()
    return nc


def _in_maps(inputs):
    A = np.asarray(inputs["A"], dtype=np.float32)
    X = np.asarray(inputs["inputs"], dtype=np.float32)
    W = np.asarray(inputs["W"], dtype=np.float32)
    b = np.asarray(inputs["b"], dtype=np.float32).reshape(OUT_F)
    a1 = np.asarray(inputs["a1"], dtype=np.float32).reshape(OUT_F)
    a2 = np.asarray(inputs["a2"], dtype=np.float32).reshape(OUT_F)

    park = np.zeros((128, 195), np.float32)
    park[:, 0:OUT_F] = W.T
    park[0:OUT_F, 64] = b
    park[0:OUT_F, 65] = a1
    park[0:OUT_F, 66] = a2
    park[0:OUT_F, 67:195] = W

    wbits = np.array(MASK8, np.uint8).reshape(4, 1, 1, 1)
    maps = []
    for c in range(NCORES):
        c0 = c * M_LOC
        stripe = A[:, c0:c0 + M_LOC]
        ash = np.concatenate([stripe[c0:], stripe[:c0]], axis=0)
        R = (ash.reshape(4, 16, 128, M_LOC) != 0)
        B = (R * wbits).sum(axis=0).astype(np.uint8)      # [16, 128, 1024]
        B = np.ascontiguousarray(B.transpose(1, 0, 2).reshape(128, 2 * N))
        xr = np.concatenate([X[c0:], X[:c0]], axis=0)
        maps.append({
            "apack": B.view(np.uint16),
            "inputst": np.ascontiguousarray(xr.T).astype(ml_dtypes.bfloat16),
            "park": park,
        })
    return maps


def _run(inputs, trace=False):
    from concourse.bass_utils import run_bass_kernel_spmd

    if "nc" not in _CACHE:
        _CACHE["nc"] = _build()
    nc = _CACHE["nc"]
    res = run_bass_kernel_spmd(nc, _in_maps(inputs), list(range(NCORES)),
                               trace=trace)
    parts = []
    for c in range(NCORES):
        packed = res.results[c]["out"]  # [128, MB*OUT_F], block-interleaved
        parts.append(packed.reshape(128, MB, OUT_F).transpose(1, 0, 2)
                     .reshape(M_LOC, OUT_F))
    out = np.concatenate(parts, axis=0)
    return out, res


def kernel(**inputs):
    out, _ = _run(inputs, trace=False)
    return out


def kernel_traced(**inputs):
    out, res = _run(inputs, trace=True)
    return out, res


# revision 13
# speedup vs baseline: 1.1081x; 1.0828x over previous
"""AAGNN attention message-passing kernel for 8 TRN2 NeuronCores.

Math: the reference builds a dense masked attention
    att = rownorm(exp(lrelu(A*zi + diag(zj))) * A);  out = relu(z - att @ z)
Since A is 0/1 with self-loops, row i of att has only two distinct values,
so with deg(i) = rowsum(A)[i], S = (deg-1)*e_off + e_diag:
    out = relu(beta*z - alpha*(A@z)),  alpha = e_off/S, beta = 1-(e_diag-e_off)/S
One pass over A (A@[z|1] giving Az and deg) is the entire memory cost.

A compression: A is 0/1, so the host packs FOUR A-bits per byte at bit
positions 5,4,3,2. On-chip, a single DVE bitwise_and with mask
(0x20>>g) per plane yields bytes whose *bit patterns* are valid fp8e4
values c_g = {2^-3, 2^-5, 2^-6, 2^-7} (times the A bit). Each group of
16 contraction slots g has its z-slot pre-scaled by s_g = {2,8,16,32}
so c_g*s_g = 1/4 uniformly: the PSUM accumulates 0.25*[Az | deg]
exactly (up to fp8 quantization of z, ~0.6% global rel err). The A
stream drops 16 MB (f32: 32 MB) -> 2 MB per core.

Matmul: fp8 DoubleRow perf mode (2 contraction slots per instruction,
0.5 cycles/row) on the TensorEngine: lhsT = [128, 2, 65] z-slot pair,
rhs = [128, 2, 512] decompressed A, out += [65, 512] per half.

z production: bias is NOT added on-chip in the hot loop; instead
z0 = x@W^T accumulates in PSUM quads (4 slots/bank), the idle
Activation engine emits fp8 z-slots with a pure scale (af=Copy), and
the epilogue adds the analytically-folded bias term
(beta - alpha*deg)*b to the output. zi/zj keep a separate f32 path.

Sharding: core c owns output rows [c*1024, (c+1)*1024); A is symmetric
so the column stripe equals the row shard transposed. Rows are rolled
by -c0 so the SPMD graph is identical across cores. No collectives.
"""

import sys

for _p in ("/opt/trn_rl_repo",):
    if _p not in sys.path:
        sys.path.insert(0, _p)

import ml_dtypes
import numpy as np

N = 8192
IN_F = 128
OUT_F = 64
NCORES = 8
M_LOC = N // NCORES      # 1024 rows per core
NT = N // 128            # 64 contraction slots
MB = M_LOC // 128        # 8 output row-blocks per core
SLOTW = 128              # z-slot width: dual-fp8 ldweights needs all 128 cols
DEGC = OUT_F + 1         # 65: one past the deg column
AZW = OUT_F + 2          # 66: Az rows + deg row (+ pad) kept from PSUM
AUGW = OUT_F + 2         # 66: z cols + zi col + zj col
HALF = M_LOC // 2        # 512: one PSUM bank of f32
NPAIR = NT // 2          # 32 DoubleRow pairs
S_G = (2.0, 8.0, 16.0, 32.0)          # z-slot scales per 16-slot group
MASK8 = (0x20, 0x10, 0x08, 0x04)      # bit-plane masks (fp8e4 c_g = 0.25/s_g)

_CACHE = {}


def _emit(tc, nc, bass, mybir, make_identity, apack, inputst, park, out_ext):
    f32 = mybir.dt.float32
    bf16 = mybir.dt.bfloat16
    f8 = mybir.dt.float8e4
    u16 = mybir.dt.uint16
    Act = mybir.ActivationFunctionType
    Alu = mybir.AluOpType
    DR = mybir.MatmulPerfMode.DoubleRow

    consts = tc.alloc_tile_pool(name="consts", bufs=1)
    decp = tc.alloc_tile_pool(name="decp", bufs=32)
    ep1 = tc.alloc_tile_pool(name="ep1", bufs=2)
    ep64 = tc.alloc_tile_pool(name="ep64", bufs=4)
    outp = tc.alloc_tile_pool(name="outp", bufs=3)

    # ---- input streaming, issued before anything else ---------------------
    park_sb = consts.tile([128, 195], f32)
    nc.sync.dma_start(out=park_sb, in_=park[:, :])
    in_all = consts.tile([IN_F, N], bf16)
    apack_sb = consts.tile([128, N], u16)   # 4 A-bits per byte
    nc.sync.dma_start(out=in_all[:, 0:512], in_=inputst[:, 0:512])
    nc.sync.dma_start(out=apack_sb[:, 0:4096], in_=apack[:, 0:4096])
    nc.sync.dma_start(out=in_all[:, 512:4096], in_=inputst[:, 512:4096])
    nc.sync.dma_start(out=apack_sb[:, 4096:8192], in_=apack[:, 4096:8192])
    nc.sync.dma_start(out=in_all[:, 4096:8192], in_=inputst[:, 4096:8192])

    # ---- constants / setup ------------------------------------------------
    identity = consts.tile([128, 128], f32)
    make_identity(nc, identity)
    ones1 = consts.tile([1, 128], f32)
    nc.gpsimd.memset(ones1, 1.0)
    warm = consts.tile([1, 1], f32)
    nc.scalar.activation(warm, ones1[0:1, 0:1], Act.Copy)

    wt_aug = consts.tile([128, AUGW], bf16)   # [W^T | w1 | w2]
    bias_bc = consts.tile([128, AUGW], f32)   # [b bcast | a1.b | a2.b]
    z8 = consts.tile([128, NT * SLOTW], f8)   # fp8 z slots [s*z | s]
    z_loc32 = consts.tile([128, MB * OUT_F], f32)
    zi_loc = consts.tile([128, 2 * MB], f32)  # local [zi | zj] per block

    # z8 pad + scaled-ones presets: no input deps, so DVE runs these at t=0.
    # Bulk pad (bytes 66..127 of each slot) as u16 for the 2x DVE mode.
    z8v = z8.rearrange("p (t w) -> p t w", w=SLOTW)
    nc.vector.memset(z8v[:, :, OUT_F + 2:SLOTW].bitcast(u16), 0)
    nc.vector.memset(z8v[:, :, DEGC:OUT_F + 2], 0.0)
    for g in range(4):
        nc.vector.memset(z8v[:, g * 16:(g + 1) * 16, OUT_F:DEGC], S_G[g])

    pre_psum = tc.alloc_tile_pool(name="pre_psum", bufs=1, space="PSUM")
    nc.vector.tensor_copy(out=wt_aug[:, 0:OUT_F], in_=park_sb[:, 0:OUT_F])
    w12_ps = pre_psum.tile([128, 2], f32)
    nc.tensor.matmul(w12_ps, lhsT=park_sb[0:OUT_F, 67:195],
                     rhs=park_sb[0:OUT_F, 65:67], start=True, stop=True)
    nc.vector.tensor_copy(out=wt_aug[:, OUT_F:AUGW], in_=w12_ps)

    brow_ps = pre_psum.tile([1, OUT_F], f32)
    nc.tensor.transpose(brow_ps, park_sb[0:OUT_F, 64:65],
                        identity[0:OUT_F, 0:OUT_F])
    ab_ps = pre_psum.tile([1, 2], f32)
    nc.tensor.matmul(ab_ps, lhsT=park_sb[0:OUT_F, 64:65],
                     rhs=park_sb[0:OUT_F, 65:67], start=True, stop=True)
    rhs_row = consts.tile([1, AUGW], f32)
    nc.vector.tensor_copy(out=rhs_row[0:1, 0:OUT_F], in_=brow_ps)
    nc.vector.tensor_copy(out=rhs_row[0:1, OUT_F:AUGW], in_=ab_ps)
    bias_ps = pre_psum.tile([128, AUGW], f32)
    nc.tensor.matmul(bias_ps, lhsT=ones1, rhs=rhs_row, start=True, stop=True)
    nc.vector.tensor_copy(out=bias_bc, in_=bias_ps)
    pre_psum.release()

    # ---- fused z production + decompress + message-passing matmul --------
    psum2 = tc.alloc_tile_pool(name="psum2", bufs=1, space="PSUM")
    acc_t = [psum2.tile([128, HALF], f32, tag=f"acct{h}", name=f"acct{h}")
             for h in range(2)]
    zpsum = tc.alloc_tile_pool(name="zpsum", bufs=3, space="PSUM")

    def z_quad(Q):
        zq = zpsum.tile([128, 4 * AUGW], f32, tag="zq", name=f"zq{Q}")
        for j in range(4):
            kb = 4 * Q + j
            nc.tensor.matmul(zq[:, j * AUGW:(j + 1) * AUGW],
                             lhsT=in_all[:, kb * 128:(kb + 1) * 128],
                             rhs=wt_aug, start=True, stop=True)
        zqv = zq.rearrange("p (j w) -> p j w", w=AUGW)
        nc.scalar.activation(z8v[:, 4 * Q:4 * Q + 4, 0:OUT_F],
                             zqv[:, :, 0:OUT_F], Act.Copy,
                             scale=float(S_G[Q // 4]))
        if Q < 2:
            zl = z_loc32.rearrange("p (j w) -> p j w", w=OUT_F)
            nc.scalar.activation(zl[:, 4 * Q:4 * Q + 4, :],
                                 zqv[:, :, 0:OUT_F], Act.Copy)
            ziv = zi_loc.rearrange("p (m t) -> p m t", t=2)
            for c in range(2):
                nc.scalar.activation(
                    ziv[:, 4 * Q:4 * Q + 4, c:c + 1],
                    zqv[:, :, OUT_F + c:OUT_F + c + 1], Act.Identity,
                    bias=bias_bc[:, OUT_F + c:OUT_F + c + 1])

    z_quad(0)
    z_quad(1)
    for t in range(NPAIR):
        if t % 2 == 0 and t // 2 + 2 < 16:
            z_quad(t // 2 + 2)
        g = t // 8
        q = (2 * t) % 16
        dec = decp.tile([128, 2 * M_LOC], f8, tag="dec", name=f"dec{t}")
        nc.vector.tensor_scalar(
            out=dec[:, :].bitcast(u16),
            in0=apack_sb[:, q * 512:q * 512 + 1024],
            scalar1=MASK8[g] * 0x101, scalar2=None, op0=Alu.bitwise_and)
        zpair = z8v[:, 2 * t:2 * t + 2, :]
        decv = dec.rearrange("p (k m) -> p k m", m=M_LOC)
        for h in range(2):
            nc.tensor.matmul(acc_t[h], lhsT=zpair,
                             rhs=decv[:, :, h * HALF:(h + 1) * HALF],
                             start=(t == 0), stop=(t == NPAIR - 1),
                             perf_mode=DR)

    # ---- batched attention-coefficient math (zi/zj part, runs early) -----
    zis = zi_loc.rearrange("p (m t) -> p m t", t=2)[:, :, 0:1]  # [128, 8, 1]
    zjs = zi_loc.rearrange("p (m t) -> p m t", t=2)[:, :, 1:2]
    s8 = ep1.tile([128, MB], f32, tag="s8")
    nc.vector.tensor_add(s8, zis, zjs)
    t8 = ep1.tile([128, MB], f32, tag="t8")
    nc.vector.tensor_scalar_mul(t8, zis, 0.01)
    l8 = ep1.tile([128, MB], f32, tag="l8")
    nc.vector.tensor_max(l8, zis, t8)
    eoff8 = ep1.tile([128, MB], f32, tag="eoff8")
    nc.scalar.activation(eoff8, l8, Act.Exp)
    t8b = ep1.tile([128, MB], f32, tag="t8b")
    nc.vector.tensor_scalar_mul(t8b, s8, 0.01)
    l8b = ep1.tile([128, MB], f32, tag="l8b")
    nc.vector.tensor_max(l8b, s8, t8b)
    ediag8 = ep1.tile([128, MB], f32, tag="ediag8")
    nc.scalar.activation(ediag8, l8b, Act.Exp)
    gd8 = ep1.tile([128, MB], f32, tag="gd8")
    nc.vector.tensor_sub(gd8, ediag8, eoff8)

    zpsum.release()

    # copy 0.25*[Az|deg]^T to SBUF; deg row first so the coefficient
    # chain (transposes + S/alpha/beta math) starts before the bulk copy
    azt_sb = consts.tile([AZW, M_LOC], f32)
    for h in range(2):
        nc.scalar.activation(azt_sb[OUT_F:DEGC, h * HALF:(h + 1) * HALF],
                             acc_t[h][OUT_F:DEGC, :], Act.Copy)
    for h in range(2):
        nc.scalar.activation(azt_sb[0:OUT_F, h * HALF:(h + 1) * HALF],
                             acc_t[h][0:OUT_F, :], Act.Copy)

    # deg row -> node-on-partition via single-column PE transposes
    tpsum = tc.alloc_tile_pool(name="tpsum", bufs=4, space="PSUM")
    dpsum = tc.alloc_tile_pool(name="dpsum", bufs=1, space="PSUM")
    tp_deg = dpsum.tile([128, MB], f32)   # = 0.25*deg
    for mb in range(MB):
        nc.tensor.transpose(tp_deg[:, mb:mb + 1],
                            azt_sb[OUT_F:DEGC, mb * 128:(mb + 1) * 128],
                            identity[OUT_F:DEGC, OUT_F:DEGC])
    dm8 = ep1.tile([128, MB], f32, tag="dm8")   # deg - 1
    nc.vector.tensor_scalar(out=dm8, in0=tp_deg, scalar1=4.0, scalar2=-1.0,
                            op0=Alu.mult, op1=Alu.add)
    S8 = ep1.tile([128, MB], f32, tag="S8")
    nc.vector.tensor_tensor(out=S8, in0=dm8, in1=eoff8, op=Alu.mult)
    nc.vector.tensor_add(S8, S8, ediag8)
    rS8 = ep1.tile([128, MB], f32, tag="rS8")
    nc.vector.reciprocal(rS8, S8)
    alpha8 = ep1.tile([128, MB], f32, tag="alpha8")   # = 4*alpha
    nc.vector.scalar_tensor_tensor(alpha8, in0=eoff8, scalar=4.0, in1=rS8,
                                   op0=Alu.mult, op1=Alu.mult)
    gamma8 = ep1.tile([128, MB], f32, tag="gamma8")
    nc.vector.tensor_mul(gamma8, gd8, rS8)
    beta8 = ep1.tile([128, MB], f32, tag="beta8")
    nc.vector.tensor_scalar(
        out=beta8, in0=gamma8, scalar1=-1.0, scalar2=1.0,
        op0=Alu.mult, op1=Alu.add)
    adeg8 = ep1.tile([128, MB], f32, tag="adeg8")   # alpha*deg
    nc.vector.tensor_mul(adeg8, alpha8, tp_deg)
    c8 = ep1.tile([128, MB], f32, tag="c8")   # beta - alpha*deg
    nc.vector.tensor_sub(c8, beta8, adeg8)

    # ---- per-block: transpose Az back, combine + bias term, relu ---------
    osb_all = outp.tile([128, MB * OUT_F], f32)
    for mb in range(MB):
        tp = tpsum.tile([128, AZW], f32, tag="tp", name=f"tp{mb}")
        nc.tensor.transpose(tp, azt_sb[:, mb * 128:(mb + 1) * 128],
                            identity[:AZW, :AZW])
        zloc = z_loc32[:, mb * OUT_F:(mb + 1) * OUT_F]
        t1 = ep64.tile([128, OUT_F], f32, tag="t1")
        nc.scalar.activation(t1, tp[:, 0:OUT_F], Act.Copy,
                             scale=alpha8[:, mb:mb + 1])
        h1 = ep64.tile([128, OUT_F], f32, tag="h1")
        nc.vector.scalar_tensor_tensor(
            h1, in0=zloc, scalar=beta8[:, mb:mb + 1], in1=t1,
            op0=Alu.mult, op1=Alu.subtract)
        h2 = ep64.tile([128, OUT_F], f32, tag="h2")
        nc.vector.scalar_tensor_tensor(
            h2, in0=bias_bc[:, 0:OUT_F], scalar=c8[:, mb:mb + 1], in1=h1,
            op0=Alu.mult, op1=Alu.add)
        nc.scalar.activation(osb_all[:, mb * OUT_F:(mb + 1) * OUT_F], h2,
                             Act.Relu)
        if mb == 3:
            nc.sync.dma_start(out=out_ext[:, 0:4 * OUT_F],
                              in_=osb_all[:, 0:4 * OUT_F])
    nc.sync.dma_start(out=out_ext[:, 4 * OUT_F:], in_=osb_all[:, 4 * OUT_F:])

    dpsum.release()
    tpsum.release()
    psum2.release()
    for p in (outp, ep64, ep1, decp, consts):
        p.release()


def _build():
    import concourse.bass as bass
    import concourse.mybir as mybir
    import concourse.tile as tile
    from concourse import bacc
    from concourse.masks import make_identity

    f32 = mybir.dt.float32
    nc = bacc.Bacc("TRN2", target_bir_lowering=False, debug=False)
    # apack: 4 A-bits per byte at bit positions 5..2; byte[p, q*1024+m] holds
    # A_rolled[(g*16+q)*128 + p, m] at bit (5-g). Viewed as uint16 on device.
    apack = nc.declare_dram_parameter("apack", [128, N], mybir.dt.uint16,
                                      isOutput=False)
    inputst = nc.declare_dram_parameter("inputst", [IN_F, N],
                                        mybir.dt.bfloat16, isOutput=False)
    # park: [W^T | b | a1 | a2] in one tensor (single param DMA)
    park = nc.declare_dram_parameter("park", [128, 195], f32,
                                     isOutput=False)
    out_ext = nc.declare_dram_parameter("out", [128, MB * OUT_F], f32,
                                        isOutput=True)

    with tile.TileContext(nc) as tc:
        _emit(tc, nc, bass, mybir, make_identity, apack, inputst, park,
              out_ext)
    nc.compile()
    return nc


def _in_maps(inputs):
    A = np.asarray(inputs["A"], dtype=np.float32)
    X = np.asarray(inputs["inputs"], dtype=np.float32)
    W = np.asarray(inputs["W"], dtype=np.float32)
    b = np.asarray(inputs["b"], dtype=np.float32).reshape(OUT_F)
    a1 = np.asarray(inputs["a1"], dtype=np.float32).reshape(OUT_F)
    a2 = np.asarray(inputs["a2"], dtype=np.float32).reshape(OUT_F)

    park = np.zeros((128, 195), np.float32)
    park[:, 0:OUT_F] = W.T
    park[0:OUT_F, 64] = b
    park[0:OUT_F, 65] = a1
    park[0:OUT_F, 66] = a2
    park[0:OUT_F, 67:195] = W

    wbits = np.array(MASK8, np.uint8).reshape(4, 1, 1, 1)
    maps = []
    for c in range(NCORES):
        c0 = c * M_LOC
        stripe = A[:, c0:c0 + M_LOC]
        ash = np.concatenate([stripe[c0:], stripe[:c0]], axis=0)
        R = (ash.reshape(4, 16, 128, M_LOC) != 0)
        B = (R * wbits).sum(axis=0).astype(np.uint8)      # [16, 128, 1024]
        B = np.ascontiguousarray(B.transpose(1, 0, 2).reshape(128, 2 * N))
        xr = np.concatenate([X[c0:], X[:c0]], axis=0)
        maps.append({
            "apack": B.view(np.uint16),
            "inputst": np.ascontiguousarray(xr.T).astype(ml_dtypes.bfloat16),
            "park": park,
        })
    return maps


def _run(inputs, trace=False):
    from concourse.bass_utils import run_bass_kernel_spmd

    if "nc" not in _CACHE:
        _CACHE["nc"] = _build()
    nc = _CACHE["nc"]
    res = run_bass_kernel_spmd(nc, _in_maps(inputs), list(range(NCORES)),
                               trace=trace)
    parts = []
    for c in range(NCORES):
        packed = res.results[c]["out"]  # [128, MB*OUT_F], block-interleaved
        parts.append(packed.reshape(128, MB, OUT_F).transpose(1, 0, 2)
                     .reshape(M_LOC, OUT_F))
    out = np.concatenate(parts, axis=0)
    return out, res


def kernel(**inputs):
    out, _ = _run(inputs, trace=False)
    return out


def kernel_traced(**inputs):
    out, res = _run(inputs, trace=True)
    return out, res
